# revision 28
# baseline (speedup 1.0000x reference)
"""CausalPrefixAttention Trainium2 Bass kernel.

Sharding: core = 4*batch + head_group. Each core computes, for its batch b and
its 4 heads, the full pipeline LN(x), LN(context) -> q/k/v projections ->
causal-prefix attention -> out @ Wo_slice, producing a [2048, 1024] partial.
Host sums the 4 partials per batch (row-parallel Wo) and adds bo.
"""

import sys

import numpy as np

for _p in ("/opt/trn_rl_repo", "/root/.axon_site/_ro/trn_rl_repo"):
    if _p not in sys.path:
        sys.path.append(_p)

import ml_dtypes  # noqa: E402

import concourse.bass as bass  # noqa: E402
import concourse.mybir as mybir  # noqa: E402
import concourse.tile as tile  # noqa: E402
from concourse import bacc  # noqa: E402
from concourse.bass_utils import run_bass_kernel_spmd  # noqa: E402

BF16 = mybir.dt.bfloat16
F32 = mybir.dt.float32
F32R = mybir.dt.float32r

N = 2048          # query tokens per batch
CTX = 2048        # context tokens per batch
DIM = 1024
DH = 64           # head dim
HPC = 4           # heads per core
CPC = HPC * DH    # 256 inner cols per core
J = CTX + N       # 4096 total keys
EPS = 1e-5

AF = mybir.ActivationFunctionType
ALU = mybir.AluOpType


def build_nc() -> bass.Bass:
    nc = bacc.Bacc()

    xb = nc.declare_dram_parameter("xb", [N, DIM], BF16, isOutput=False)
    cb = nc.declare_dram_parameter("cb", [CTX, DIM], BF16, isOutput=False)
    wq = nc.declare_dram_parameter("wq", [DIM, CPC], BF16, isOutput=False)
    wk = nc.declare_dram_parameter("wk", [DIM, CPC], BF16, isOutput=False)
    wv = nc.declare_dram_parameter("wv", [DIM, CPC], BF16, isOutput=False)
    wo = nc.declare_dram_parameter("wo", [64, HPC, DIM], BF16, isOutput=False)
    g1v = nc.declare_dram_parameter("g1v", [128, 8], F32, isOutput=False)
    b1v = nc.declare_dram_parameter("b1v", [128, 8], F32, isOutput=False)
    g2v = nc.declare_dram_parameter("g2v", [128, 8], F32, isOutput=False)
    b2v = nc.declare_dram_parameter("b2v", [128, 8], F32, isOutput=False)
    cmv = nc.declare_dram_parameter("cmv", [128, 16], F32, isOutput=False)
    tri01 = nc.declare_dram_parameter("tri01", [128, 128], BF16, isOutput=False)
    ident = nc.declare_dram_parameter("ident", [128, 128], BF16, isOutput=False)
    sel16 = nc.declare_dram_parameter("sel16", [16, 2048], F32, isOutput=False)
    out_d = nc.declare_dram_parameter("out", [N, DIM], F32, isOutput=True)

    with tile.TileContext(nc) as tc:
        with (
            tc.tile_pool(name="singles", bufs=1) as singles,
            tc.tile_pool(name="acts", bufs=1) as acts,
            tc.tile_pool(name="ln", bufs=3) as ln_pool,
            tc.tile_pool(name="lns", bufs=4) as lns,
            tc.tile_pool(name="es", bufs=3) as es_pool,
            tc.tile_pool(name="dstg", bufs=3) as dstg_pool,
            tc.tile_pool(name="outp", bufs=2) as out_pool,
            tc.tile_pool(name="ps", bufs=2, space="PSUM") as psum,
            tc.tile_pool(name="dnp", bufs=1, space="PSUM") as den_pool,
            tc.tile_pool(name="avps", bufs=3, space="PSUM") as av_pool,
        ):
            # --- constants / weights to SBUF ---
            wq_sb = singles.tile([128, 8, CPC], BF16)
            nc.sync.dma_start(wq_sb, wq.rearrange("(t p) c -> p t c", p=128))
            wk_sb = singles.tile([128, 8, CPC], BF16)
            nc.sync.dma_start(wk_sb, wk.rearrange("(t p) c -> p t c", p=128))
            wv_sb = singles.tile([128, 8, CPC], BF16)
            nc.sync.dma_start(wv_sb, wv.rearrange("(t p) c -> p t c", p=128))
            wo_sb = singles.tile([64, HPC, DIM], BF16)
            nc.sync.dma_start(wo_sb, wo[:])
            scol = singles.tile([128, 16, 16], F32)
            nc.vector.memset(scol, 0.0)
            for r in range(16):
                nc.vector.memset(scol[64:65, r, r:r + 1], 1.0)
            g1_sb = singles.tile([128, 8], F32)
            nc.sync.dma_start(g1_sb, g1v[:])
            b1_sb = singles.tile([128, 8], F32)
            nc.sync.dma_start(b1_sb, b1v[:])
            g2_sb = singles.tile([128, 8], F32)
            nc.sync.dma_start(g2_sb, g2v[:])
            b2_sb = singles.tile([128, 8], F32)
            nc.sync.dma_start(b2_sb, b2v[:])
            cm_sb = singles.tile([128, 16], F32)
            nc.sync.dma_start(cm_sb, cmv[:])
            tri_sb = singles.tile([128, 128], BF16)
            nc.sync.dma_start(tri_sb, tri01[:])
            id_sb = singles.tile([128, 128], BF16)
            nc.sync.dma_start(id_sb, ident[:])
            sel_sb = singles.tile([16, 2048], F32)
            nc.sync.dma_start(sel_sb, sel16[:])
            eps_sb = singles.tile([128, 1], F32)
            nc.vector.memset(eps_sb, EPS)

            # --- LayerNorm + transpose: [tokens, DIM] -> [128, 8, tokens] ---
            # stats on ScalarE via accum_out; apply + transpose-copy on DVE
            def layernorm_T(src_dram, g_sb, b_sb, ntok, name):
                dstT = acts.tile([128, 8, ntok], BF16, tag=f"T{name}")
                for rt in range(ntok // 128):
                    xt = ln_pool.tile([128, DIM], BF16, tag="xt")
                    nc.sync.dma_start(xt, src_dram[rt * 128:(rt + 1) * 128, :])
                    st = lns.tile([128, 2, 6], F32, tag="st")
                    nc.vector.bn_stats(st[:, 0, :], xt[:, 0:512])
                    nc.vector.bn_stats(st[:, 1, :], xt[:, 512:1024])
                    mv = lns.tile([128, 2], F32, tag="mv")
                    nc.vector.bn_aggr(mv, st)
                    std = lns.tile([128, 1], F32, tag="std")
                    nc.scalar.activation(std, mv[:, 1:2], AF.Sqrt, bias=eps_sb)
                    rstd = lns.tile([128, 1], F32, tag="rstd")
                    nc.vector.reciprocal(rstd, std)
                    xn = ln_pool.tile([128, DIM], BF16, tag="xn")
                    nc.vector.tensor_scalar(
                        xn, xt, mv[:, 0:1], rstd, op0=ALU.subtract, op1=ALU.mult
                    )
                    for fg in range(2):
                        pst = av_pool.tile([128, 512], BF16, tag="av")
                        for k in range(4):
                            ft = fg * 4 + k
                            nc.tensor.transpose(
                                pst[:, k * 128:(k + 1) * 128],
                                xn[:, ft * 128:(ft + 1) * 128],
                                id_sb,
                            )
                        nc.scalar.copy(
                            dstT[:, fg * 4:(fg + 1) * 4, rt * 128:(rt + 1) * 128],
                            pst.rearrange("p (f c) -> p f c", f=4),
                        )
                # gamma/beta are per-feature = per-partition scalars here
                for ft in range(8):
                    nc.vector.tensor_scalar(
                        dstT[:, ft, :], dstT[:, ft, :],
                        g_sb[:, ft:ft + 1], b_sb[:, ft:ft + 1],
                        op0=ALU.mult, op1=ALU.add,
                    )
                return dstT

            xnT = layernorm_T(xb, g1_sb, b1_sb, N, "x")
            cnT = layernorm_T(cb, g2_sb, b2_sb, CTX, "c")

            # --- projections, split per c-tile so attention on heads 0/1 can
            # overlap with the projections for heads 2/3 ---
            def make_qT(ct):
                qT = acts.tile([128, N], BF16, tag=f"qT{ct}")
                for it in range(N // 1024):
                    ps = psum.tile([128, 1024], F32, tag="ps")
                    for half in range(2):
                        off = it * 1024 + half * 512
                        for kt in range(8):
                            nc.tensor.matmul(
                                ps[:, half * 512:(half + 1) * 512],
                                wq_sb[:, kt, ct * 128:(ct + 1) * 128],
                                xnT[:, kt, off:off + 512],
                                start=(kt == 0), stop=(kt == 7),
                            )
                    nc.vector.tensor_copy(qT[:, it * 1024:(it + 1) * 1024], ps)
                return qT

            def make_kT(ct):
                kT = acts.tile([128, J], BF16, tag=f"kT{ct}")
                for jt in range(J // 1024):
                    ps = psum.tile([128, 1024], F32, tag="ps")
                    for half in range(2):
                        j5 = jt * 2 + half
                        srcT = cnT if j5 < 4 else xnT
                        off = (j5 % 4) * 512
                        for kt in range(8):
                            nc.tensor.matmul(
                                ps[:, half * 512:(half + 1) * 512],
                                wk_sb[:, kt, ct * 128:(ct + 1) * 128],
                                srcT[:, kt, off:off + 512],
                                start=(kt == 0), stop=(kt == 7),
                            )
                    nc.vector.tensor_copy(kT[:, jt * 1024:(jt + 1) * 1024], ps)
                return kT

            qTs = {0: make_qT(0)}
            kTs = {0: make_kT(0)}

            # --- v natural [keys, 4 heads, 64+aug] ---
            v_sb = acts.tile([128, 32, HPC, 66], BF16)
            for jb in range(32):
                srcT = cnT if jb < 16 else xnT
                off = (jb % 16) * 128
                ps = psum.tile([128, 1024], F32, tag="ps")
                for kt in range(8):
                    nc.tensor.matmul(
                        ps[:, 0:CPC],
                        srcT[:, kt, off:off + 128],
                        wv_sb[:, kt, :],
                        start=(kt == 0), stop=(kt == 7),
                    )
                nc.vector.tensor_copy(
                    v_sb[:, jb, :, 0:64],
                    ps[:, 0:CPC].rearrange("p (h d) -> p h d", h=HPC),
                )
                if jb < 16:
                    # context_mask: zero masked rows, aug col = mask
                    nc.vector.tensor_scalar_mul(
                        v_sb[:, jb, :, 0:64], v_sb[:, jb, :, 0:64],
                        cm_sb[:, jb:jb + 1],
                    )
                    nc.vector.tensor_copy(
                        v_sb[:, jb, :, 64:65],
                        cm_sb[:, jb:jb + 1, None].to_broadcast((128, HPC, 1)),
                    )
                else:
                    nc.vector.memset(v_sb[:, jb, :, 64:65], 1.0)

            # --- attention ---
            rden = singles.tile([16, 512], F32)
            # out^T as 16 separate tiles (head, 512-query block) so the out
            # projection can start per-block as soon as normalization lands
            oThs = {}
            for h in range(HPC):
                for q in range(4):
                    oThs[(h, q)] = acts.tile([128, 512], BF16, tag=f"oT{h}_{q}", name=f"oT{h}_{q}")
            den_acc = den_pool.tile([8, 512], F32)
            n_den = [0, 0]

            def attend(h):
                ct, pb = h // 2, (h % 2) * 64
                kT, qT = kTs[ct], qTs[ct]
                for it in range(2):
                    i0 = it * 1024
                    njs = 16 + it * 8 + 8
                    jl0 = [j for j in range(njs)
                           if j < 16 or (j - 16) * 128 - i0 < 512]
                    jl1 = list(range(njs))
                    av0 = av_pool.tile([128, 512], F32, tag="av")
                    av1 = av_pool.tile([128, 512], F32, tag="av")
                    for jb in range(njs):
                        jj0 = (jb - 16) * 128
                        d = jj0 - i0
                        crossing = jb >= 16 and d >= 0
                        c0 = d if (crossing and d > 0) else 0
                        ps = psum.tile([128, 1024], F32, tag="ps")
                        if c0 < 512:
                            nc.tensor.matmul(
                                ps[:, c0:512],
                                kT[pb:pb + 64, jb * 128:(jb + 1) * 128],
                                qT[pb:pb + 64, i0 + c0:i0 + 512],
                                start=True, stop=True,
                            )
                        nc.tensor.matmul(
                            ps[:, max(512, c0):1024],
                            kT[pb:pb + 64, jb * 128:(jb + 1) * 128],
                            qT[pb:pb + 64, i0 + max(512, c0):i0 + 1024],
                            start=True, stop=True,
                        )
                        es = es_pool.tile([128, 1024], BF16, tag="es")
                        if c0 > 0:
                            nc.gpsimd.memset(es[:, 0:c0], 0.0)
                        nc.scalar.activation(
                            es[:, c0:1024], ps[:, c0:1024], AF.Exp)
                        if crossing:
                            nc.gpsimd.tensor_mul(
                                es[:, d:d + 128], es[:, d:d + 128], tri_sb
                            )
                        if jb in jl0:
                            nc.tensor.matmul(
                                av0[0:65, :],
                                v_sb[:, jb, h, 0:65],
                                es[:, 0:512],
                                start=(jb == jl0[0]), stop=(jb == jl0[-1]),
                            )
                        nc.tensor.matmul(
                            av1[0:65, :],
                            v_sb[:, jb, h, 0:65],
                            es[:, 512:1024],
                            start=(jb == jl1[0]), stop=(jb == jl1[-1]),
                        )
                    for half, av in ((0, av0), (1, av1)):
                        i5 = i0 + half * 512
                        nc.vector.tensor_copy(
                            oThs[(h, i5 // 512)][0:64, :], av[0:64, :])
                        dstg = dstg_pool.tile([128, 512], F32, tag="dstg")
                        nc.vector.tensor_copy(dstg[64:65, :], av[64:65, :])
                        r = h * 4 + it * 2 + half
                        grp = r // 8
                        n_den[grp] += 1
                        nc.tensor.matmul(
                            den_acc,
                            scol[64:65, r, grp * 8:(grp + 1) * 8],
                            dstg[64:65, :],
                            start=(n_den[grp] == 1), stop=(n_den[grp] == 8),
                        )

            def normalize(heads, rd):
                for it in range(4):
                    for h in heads:
                        r = (h % 2) * 4 + it
                        bc = av_pool.tile([128, 512], F32, tag="av")
                        nc.tensor.matmul(
                            bc,
                            sel_sb[0:8, r * 128:(r + 1) * 128],
                            rd,
                            start=True, stop=True,
                        )
                        nc.vector.tensor_mul(
                            oThs[(h, it)][0:64, :],
                            oThs[(h, it)][0:64, :], bc[0:64, :],
                        )

            attend(0)
            qTs[1] = make_qT(1)
            kTs[1] = make_kT(1)
            attend(1)
            nc.vector.reciprocal(rden[0:8, :], den_acc)
            normalize([0, 1], rden[0:8, :])
            attend(2)
            attend(3)
            rden2 = singles.tile([8, 512], F32)
            nc.vector.reciprocal(rden2[0:8, :], den_acc)
            normalize([2, 3], rden2[0:8, :])

            # --- out projection: out[i, :] = oTh^T @ wo (4 heads, K=64) ---
            for ib in range(N // 128):
                ot = out_pool.tile([128, DIM], F32, tag="ot")
                ps = psum.tile([128, 1024], F32, tag="ps")
                for oc in range(2):
                    for h in range(HPC):
                        nc.tensor.matmul(
                            ps[:, oc * 512:(oc + 1) * 512],
                            oThs[(h, ib // 4)][0:64,
                                               (ib % 4) * 128:(ib % 4 + 1) * 128],
                            wo_sb[:, h, oc * 512:(oc + 1) * 512],
                            start=(h == 0), stop=(h == 3),
                        )
                nc.vector.tensor_copy(ot, ps)
                nc.sync.dma_start(out_d[ib * 128:(ib + 1) * 128, :], ot)

    nc.finalize()
    return nc


def _sel16():
    s = np.zeros((16, 2048), np.float32)
    for r in range(16):
        s[r, r * 128:(r + 1) * 128] = 1.0
    return s


def make_in_maps(x, context, context_mask, g1, b1, g2, b2, Wq, Wkv, Wo):
    bf = ml_dtypes.bfloat16
    Wk = Wkv[:, :DIM]
    Wv = Wkv[:, DIM:]
    scale = DH ** -0.5
    tri = np.triu(np.ones((128, 128), np.float32)).astype(bf)

    def vec128(v):
        return np.ascontiguousarray(
            np.asarray(v, np.float32).reshape(8, 128).T
        )

    in_maps = []
    for core in range(8):
        b, g = core // 4, core % 4
        hs = slice(g * CPC, (g + 1) * CPC)
        in_maps.append(dict(
            xb=np.ascontiguousarray(x[b]).astype(bf),
            cb=np.ascontiguousarray(context[b]).astype(bf),
            wq=np.ascontiguousarray(Wq[:, hs] * scale).astype(bf),
            wk=np.ascontiguousarray(Wk[:, hs]).astype(bf),
            wv=np.ascontiguousarray(Wv[:, hs]).astype(bf),
            wo=np.ascontiguousarray(
                Wo[hs, :].reshape(HPC, 64, DIM).transpose(1, 0, 2)
            ).astype(bf),
            g1v=vec128(g1), b1v=vec128(b1), g2v=vec128(g2), b2v=vec128(b2),
            cmv=np.ascontiguousarray(
                np.asarray(context_mask[b], np.float32).reshape(16, 128).T
            ),
            tri01=tri,
            ident=np.eye(128, dtype=np.float32).astype(bf),
            sel16=_sel16(),
        ))
    return in_maps


_NC_CACHE = None


def kernel(**inputs) -> np.ndarray:
    global _NC_CACHE
    x = np.asarray(inputs["x"], np.float32)
    context = np.asarray(inputs["context"], np.float32)
    cm = np.asarray(inputs["context_mask"])
    g1 = np.asarray(inputs["g1"], np.float32)
    b1 = np.asarray(inputs["b1"], np.float32)
    g2 = np.asarray(inputs["g2"], np.float32)
    b2 = np.asarray(inputs["b2"], np.float32)
    Wq = np.asarray(inputs["Wq"], np.float32)
    Wkv = np.asarray(inputs["Wkv"], np.float32)
    Wo = np.asarray(inputs["Wo"], np.float32)
    bo = np.asarray(inputs["bo"], np.float32)

    if _NC_CACHE is None:
        _NC_CACHE = build_nc()
    nc = _NC_CACHE

    in_maps = make_in_maps(x, context, cm, g1, b1, g2, b2, Wq, Wkv, Wo)
    res = run_bass_kernel_spmd(nc, in_maps, core_ids=list(range(8))).results

    out = np.zeros((2, N, DIM), np.float32)
    for core in range(8):
        out[core // 4] += np.asarray(res[core]["out"], np.float32)
    out += bo
    return out


# revision 30
# speedup vs baseline: 1.0170x; 1.0170x over previous
"""CausalPrefixAttention Trainium2 Bass kernel.

Sharding: core = 4*batch + head_group. Each core computes, for its batch b and
its 4 heads, the full pipeline LN(x), LN(context) -> q/k/v projections ->
causal-prefix attention -> out @ Wo_slice, producing a [2048, 1024] partial.
Host sums the 4 partials per batch (row-parallel Wo) and adds bo.
"""

import sys

import numpy as np

for _p in ("/opt/trn_rl_repo", "/root/.axon_site/_ro/trn_rl_repo"):
    if _p not in sys.path:
        sys.path.append(_p)

import ml_dtypes  # noqa: E402

import concourse.bass as bass  # noqa: E402
import concourse.mybir as mybir  # noqa: E402
import concourse.tile as tile  # noqa: E402
from concourse import bacc  # noqa: E402
from concourse.bass_utils import run_bass_kernel_spmd  # noqa: E402

BF16 = mybir.dt.bfloat16
F32 = mybir.dt.float32
F32R = mybir.dt.float32r

N = 2048          # query tokens per batch
CTX = 2048        # context tokens per batch
DIM = 1024
DH = 64           # head dim
HPC = 4           # heads per core
CPC = HPC * DH    # 256 inner cols per core
J = CTX + N       # 4096 total keys
EPS = 1e-5

AF = mybir.ActivationFunctionType
ALU = mybir.AluOpType


def build_nc() -> bass.Bass:
    nc = bacc.Bacc()

    xb = nc.declare_dram_parameter("xb", [N, DIM], BF16, isOutput=False)
    cb = nc.declare_dram_parameter("cb", [CTX, DIM], BF16, isOutput=False)
    wq = nc.declare_dram_parameter("wq", [DIM, CPC], BF16, isOutput=False)
    wk = nc.declare_dram_parameter("wk", [DIM, CPC], BF16, isOutput=False)
    wv = nc.declare_dram_parameter("wv", [DIM, CPC], BF16, isOutput=False)
    wo = nc.declare_dram_parameter("wo", [64, HPC, DIM], BF16, isOutput=False)
    g1v = nc.declare_dram_parameter("g1v", [128, 8], F32, isOutput=False)
    b1v = nc.declare_dram_parameter("b1v", [128, 8], F32, isOutput=False)
    g2v = nc.declare_dram_parameter("g2v", [128, 8], F32, isOutput=False)
    b2v = nc.declare_dram_parameter("b2v", [128, 8], F32, isOutput=False)
    cmv = nc.declare_dram_parameter("cmv", [128, 16], F32, isOutput=False)
    tri01 = nc.declare_dram_parameter("tri01", [128, 128], BF16, isOutput=False)
    ident = nc.declare_dram_parameter("ident", [128, 128], BF16, isOutput=False)
    sel16 = nc.declare_dram_parameter("sel16", [16, 2048], F32, isOutput=False)
    out_d = nc.declare_dram_parameter("out", [N, DIM], F32, isOutput=True)

    with tile.TileContext(nc) as tc:
        with (
            tc.tile_pool(name="singles", bufs=1) as singles,
            tc.tile_pool(name="acts", bufs=1) as acts,
            tc.tile_pool(name="ln", bufs=3) as ln_pool,
            tc.tile_pool(name="lns", bufs=4) as lns,
            tc.tile_pool(name="es", bufs=4) as es_pool,
            tc.tile_pool(name="dstg", bufs=3) as dstg_pool,
            tc.tile_pool(name="outp", bufs=2) as out_pool,
            tc.tile_pool(name="ps", bufs=2, space="PSUM") as psum,
            tc.tile_pool(name="dnp", bufs=1, space="PSUM") as den_pool,
            tc.tile_pool(name="avps", bufs=3, space="PSUM") as av_pool,
        ):
            # --- constants / weights to SBUF ---
            wq_sb = singles.tile([128, 8, CPC], BF16)
            nc.sync.dma_start(wq_sb, wq.rearrange("(t p) c -> p t c", p=128))
            wk_sb = singles.tile([128, 8, CPC], BF16)
            nc.sync.dma_start(wk_sb, wk.rearrange("(t p) c -> p t c", p=128))
            wv_sb = singles.tile([128, 8, CPC], BF16)
            nc.sync.dma_start(wv_sb, wv.rearrange("(t p) c -> p t c", p=128))
            wo_sb = singles.tile([64, HPC, DIM], BF16)
            nc.sync.dma_start(wo_sb, wo[:])
            scol = singles.tile([128, 16, 16], F32)
            nc.vector.memset(scol, 0.0)
            for r in range(16):
                nc.vector.memset(scol[64:65, r, r:r + 1], 1.0)
            g1_sb = singles.tile([128, 8], F32)
            nc.sync.dma_start(g1_sb, g1v[:])
            b1_sb = singles.tile([128, 8], F32)
            nc.sync.dma_start(b1_sb, b1v[:])
            g2_sb = singles.tile([128, 8], F32)
            nc.sync.dma_start(g2_sb, g2v[:])
            b2_sb = singles.tile([128, 8], F32)
            nc.sync.dma_start(b2_sb, b2v[:])
            cm_sb = singles.tile([128, 16], F32)
            nc.sync.dma_start(cm_sb, cmv[:])
            tri_sb = singles.tile([128, 128], BF16)
            nc.sync.dma_start(tri_sb, tri01[:])
            id_sb = singles.tile([128, 128], BF16)
            nc.sync.dma_start(id_sb, ident[:])
            sel_sb = singles.tile([16, 2048], F32)
            nc.sync.dma_start(sel_sb, sel16[:])
            eps_sb = singles.tile([128, 1], F32)
            nc.vector.memset(eps_sb, EPS)

            # --- LayerNorm + transpose: [tokens, DIM] -> [128, 8, tokens] ---
            # stats on ScalarE via accum_out; apply + transpose-copy on DVE
            def layernorm_T(src_dram, g_sb, b_sb, ntok, name):
                dstT = acts.tile([128, 8, ntok], BF16, tag=f"T{name}")
                for rt in range(ntok // 128):
                    xt = ln_pool.tile([128, DIM], BF16, tag="xt")
                    nc.sync.dma_start(xt, src_dram[rt * 128:(rt + 1) * 128, :])
                    st = lns.tile([128, 2, 6], F32, tag="st")
                    nc.vector.bn_stats(st[:, 0, :], xt[:, 0:512])
                    nc.vector.bn_stats(st[:, 1, :], xt[:, 512:1024])
                    mv = lns.tile([128, 2], F32, tag="mv")
                    nc.vector.bn_aggr(mv, st)
                    std = lns.tile([128, 1], F32, tag="std")
                    nc.scalar.activation(std, mv[:, 1:2], AF.Sqrt, bias=eps_sb)
                    rstd = lns.tile([128, 1], F32, tag="rstd")
                    nc.vector.reciprocal(rstd, std)
                    xn = ln_pool.tile([128, DIM], BF16, tag="xn")
                    nc.vector.tensor_scalar(
                        xn, xt, mv[:, 0:1], rstd, op0=ALU.subtract, op1=ALU.mult
                    )
                    for fg in range(2):
                        pst = av_pool.tile([128, 512], BF16, tag="av")
                        for k in range(4):
                            ft = fg * 4 + k
                            nc.tensor.transpose(
                                pst[:, k * 128:(k + 1) * 128],
                                xn[:, ft * 128:(ft + 1) * 128],
                                id_sb,
                            )
                        nc.scalar.copy(
                            dstT[:, fg * 4:(fg + 1) * 4, rt * 128:(rt + 1) * 128],
                            pst.rearrange("p (f c) -> p f c", f=4),
                        )
                # gamma/beta are per-feature = per-partition scalars here
                for ft in range(8):
                    nc.vector.tensor_scalar(
                        dstT[:, ft, :], dstT[:, ft, :],
                        g_sb[:, ft:ft + 1], b_sb[:, ft:ft + 1],
                        op0=ALU.mult, op1=ALU.add,
                    )
                return dstT

            xnT = layernorm_T(xb, g1_sb, b1_sb, N, "x")
            cnT = layernorm_T(cb, g2_sb, b2_sb, CTX, "c")

            # --- projections, split per c-tile so attention on heads 0/1 can
            # overlap with the projections for heads 2/3 ---
            def make_qT(ct):
                qT = acts.tile([128, N], BF16, tag=f"qT{ct}")
                for it in range(N // 1024):
                    ps = psum.tile([128, 1024], F32, tag="ps")
                    for half in range(2):
                        off = it * 1024 + half * 512
                        for kt in range(8):
                            nc.tensor.matmul(
                                ps[:, half * 512:(half + 1) * 512],
                                wq_sb[:, kt, ct * 128:(ct + 1) * 128],
                                xnT[:, kt, off:off + 512],
                                start=(kt == 0), stop=(kt == 7),
                            )
                    nc.scalar.copy(qT[:, it * 1024:(it + 1) * 1024], ps)
                return qT

            def make_kT(ct):
                kT = acts.tile([128, J], BF16, tag=f"kT{ct}")
                for jt in range(J // 1024):
                    ps = psum.tile([128, 1024], F32, tag="ps")
                    for half in range(2):
                        j5 = jt * 2 + half
                        srcT = cnT if j5 < 4 else xnT
                        off = (j5 % 4) * 512
                        for kt in range(8):
                            nc.tensor.matmul(
                                ps[:, half * 512:(half + 1) * 512],
                                wk_sb[:, kt, ct * 128:(ct + 1) * 128],
                                srcT[:, kt, off:off + 512],
                                start=(kt == 0), stop=(kt == 7),
                            )
                    nc.scalar.copy(kT[:, jt * 1024:(jt + 1) * 1024], ps)
                return kT

            qTs = {0: make_qT(0)}
            kTs = {0: make_kT(0)}

            # --- v natural [keys, 4 heads, 64+aug] ---
            v_sb = acts.tile([128, 32, HPC, 66], BF16)
            for jb in range(32):
                srcT = cnT if jb < 16 else xnT
                off = (jb % 16) * 128
                ps = psum.tile([128, 1024], F32, tag="ps")
                for kt in range(8):
                    nc.tensor.matmul(
                        ps[:, 0:CPC],
                        srcT[:, kt, off:off + 128],
                        wv_sb[:, kt, :],
                        start=(kt == 0), stop=(kt == 7),
                    )
                nc.vector.tensor_copy(
                    v_sb[:, jb, :, 0:64],
                    ps[:, 0:CPC].rearrange("p (h d) -> p h d", h=HPC),
                )
                if jb < 16:
                    # context_mask: zero masked rows, aug col = mask
                    nc.vector.tensor_scalar_mul(
                        v_sb[:, jb, :, 0:64], v_sb[:, jb, :, 0:64],
                        cm_sb[:, jb:jb + 1],
                    )
                    nc.vector.tensor_copy(
                        v_sb[:, jb, :, 64:65],
                        cm_sb[:, jb:jb + 1, None].to_broadcast((128, HPC, 1)),
                    )
                else:
                    nc.vector.memset(v_sb[:, jb, :, 64:65], 1.0)

            # --- attention ---
            rden = singles.tile([16, 512], F32)
            # out^T as 16 separate tiles (head, 512-query block) so the out
            # projection can start per-block as soon as normalization lands
            oThs = {}
            for h in range(HPC):
                for q in range(4):
                    oThs[(h, q)] = acts.tile([128, 512], BF16, tag=f"oT{h}_{q}", name=f"oT{h}_{q}")
            den_acc = den_pool.tile([8, 512], F32)
            n_den = [0, 0]

            def attend(h):
                ct, pb = h // 2, (h % 2) * 64
                kT, qT = kTs[ct], qTs[ct]
                for it in range(2):
                    i0 = it * 1024
                    njs = 16 + it * 8 + 8
                    jl0 = [j for j in range(njs)
                           if j < 16 or (j - 16) * 128 - i0 < 512]
                    jl1 = list(range(njs))
                    av0 = av_pool.tile([128, 512], F32, tag="av")
                    av1 = av_pool.tile([128, 512], F32, tag="av")
                    for jb in range(njs):
                        jj0 = (jb - 16) * 128
                        d = jj0 - i0
                        crossing = jb >= 16 and d >= 0
                        c0 = d if (crossing and d > 0) else 0
                        ps = psum.tile([128, 1024], F32, tag="ps")
                        if c0 < 512:
                            nc.tensor.matmul(
                                ps[:, c0:512],
                                kT[pb:pb + 64, jb * 128:(jb + 1) * 128],
                                qT[pb:pb + 64, i0 + c0:i0 + 512],
                                start=True, stop=True,
                            )
                        nc.tensor.matmul(
                            ps[:, max(512, c0):1024],
                            kT[pb:pb + 64, jb * 128:(jb + 1) * 128],
                            qT[pb:pb + 64, i0 + max(512, c0):i0 + 1024],
                            start=True, stop=True,
                        )
                        es = es_pool.tile([128, 1024], BF16, tag="es")
                        if c0 > 0:
                            nc.gpsimd.memset(es[:, 0:c0], 0.0)
                        nc.scalar.activation(
                            es[:, c0:1024], ps[:, c0:1024], AF.Exp)
                        if crossing:
                            nc.gpsimd.tensor_mul(
                                es[:, d:d + 128], es[:, d:d + 128], tri_sb
                            )
                        if jb in jl0:
                            nc.tensor.matmul(
                                av0[0:65, :],
                                v_sb[:, jb, h, 0:65],
                                es[:, 0:512],
                                start=(jb == jl0[0]), stop=(jb == jl0[-1]),
                            )
                        nc.tensor.matmul(
                            av1[0:65, :],
                            v_sb[:, jb, h, 0:65],
                            es[:, 512:1024],
                            start=(jb == jl1[0]), stop=(jb == jl1[-1]),
                        )
                    for half, av in ((0, av0), (1, av1)):
                        i5 = i0 + half * 512
                        nc.vector.tensor_copy(
                            oThs[(h, i5 // 512)][0:64, :], av[0:64, :])
                        dstg = dstg_pool.tile([128, 512], F32, tag="dstg")
                        nc.vector.tensor_copy(dstg[64:65, :], av[64:65, :])
                        r = h * 4 + it * 2 + half
                        grp = r // 8
                        n_den[grp] += 1
                        nc.tensor.matmul(
                            den_acc,
                            scol[64:65, r, grp * 8:(grp + 1) * 8],
                            dstg[64:65, :],
                            start=(n_den[grp] == 1), stop=(n_den[grp] == 8),
                        )

            def normalize(heads, rd):
                for it in range(4):
                    for h in heads:
                        r = (h % 2) * 4 + it
                        bc = av_pool.tile([128, 512], F32, tag="av")
                        nc.tensor.matmul(
                            bc,
                            sel_sb[0:8, r * 128:(r + 1) * 128],
                            rd,
                            start=True, stop=True,
                        )
                        nc.vector.tensor_mul(
                            oThs[(h, it)][0:64, :],
                            oThs[(h, it)][0:64, :], bc[0:64, :],
                        )

            attend(0)
            qTs[1] = make_qT(1)
            kTs[1] = make_kT(1)
            attend(1)
            nc.vector.reciprocal(rden[0:8, :], den_acc)
            normalize([0, 1], rden[0:8, :])
            attend(2)
            attend(3)
            rden2 = singles.tile([8, 512], F32)
            nc.vector.reciprocal(rden2[0:8, :], den_acc)
            normalize([2, 3], rden2[0:8, :])

            # --- out projection: out[i, :] = oTh^T @ wo (4 heads, K=64) ---
            for ib in range(N // 128):
                ot = out_pool.tile([128, DIM], F32, tag="ot")
                ps = psum.tile([128, 1024], F32, tag="ps")
                for oc in range(2):
                    for h in range(HPC):
                        nc.tensor.matmul(
                            ps[:, oc * 512:(oc + 1) * 512],
                            oThs[(h, ib // 4)][0:64,
                                               (ib % 4) * 128:(ib % 4 + 1) * 128],
                            wo_sb[:, h, oc * 512:(oc + 1) * 512],
                            start=(h == 0), stop=(h == 3),
                        )
                nc.scalar.copy(ot, ps)
                nc.sync.dma_start(out_d[ib * 128:(ib + 1) * 128, :], ot)

    nc.finalize()
    return nc


def _sel16():
    s = np.zeros((16, 2048), np.float32)
    for r in range(16):
        s[r, r * 128:(r + 1) * 128] = 1.0
    return s


def make_in_maps(x, context, context_mask, g1, b1, g2, b2, Wq, Wkv, Wo):
    bf = ml_dtypes.bfloat16
    Wk = Wkv[:, :DIM]
    Wv = Wkv[:, DIM:]
    scale = DH ** -0.5
    tri = np.triu(np.ones((128, 128), np.float32)).astype(bf)

    def vec128(v):
        return np.ascontiguousarray(
            np.asarray(v, np.float32).reshape(8, 128).T
        )

    in_maps = []
    for core in range(8):
        b, g = core // 4, core % 4
        hs = slice(g * CPC, (g + 1) * CPC)
        in_maps.append(dict(
            xb=np.ascontiguousarray(x[b]).astype(bf),
            cb=np.ascontiguousarray(context[b]).astype(bf),
            wq=np.ascontiguousarray(Wq[:, hs] * scale).astype(bf),
            wk=np.ascontiguousarray(Wk[:, hs]).astype(bf),
            wv=np.ascontiguousarray(Wv[:, hs]).astype(bf),
            wo=np.ascontiguousarray(
                Wo[hs, :].reshape(HPC, 64, DIM).transpose(1, 0, 2)
            ).astype(bf),
            g1v=vec128(g1), b1v=vec128(b1), g2v=vec128(g2), b2v=vec128(b2),
            cmv=np.ascontiguousarray(
                np.asarray(context_mask[b], np.float32).reshape(16, 128).T
            ),
            tri01=tri,
            ident=np.eye(128, dtype=np.float32).astype(bf),
            sel16=_sel16(),
        ))
    return in_maps


_NC_CACHE = None


def kernel(**inputs) -> np.ndarray:
    global _NC_CACHE
    x = np.asarray(inputs["x"], np.float32)
    context = np.asarray(inputs["context"], np.float32)
    cm = np.asarray(inputs["context_mask"])
    g1 = np.asarray(inputs["g1"], np.float32)
    b1 = np.asarray(inputs["b1"], np.float32)
    g2 = np.asarray(inputs["g2"], np.float32)
    b2 = np.asarray(inputs["b2"], np.float32)
    Wq = np.asarray(inputs["Wq"], np.float32)
    Wkv = np.asarray(inputs["Wkv"], np.float32)
    Wo = np.asarray(inputs["Wo"], np.float32)
    bo = np.asarray(inputs["bo"], np.float32)

    if _NC_CACHE is None:
        _NC_CACHE = build_nc()
    nc = _NC_CACHE

    in_maps = make_in_maps(x, context, cm, g1, b1, g2, b2, Wq, Wkv, Wo)
    res = run_bass_kernel_spmd(nc, in_maps, core_ids=list(range(8))).results

    out = np.zeros((2, N, DIM), np.float32)
    for core in range(8):
        out[core // 4] += np.asarray(res[core]["out"], np.float32)
    out += bo
    return out


# revision 36
# speedup vs baseline: 1.0421x; 1.0247x over previous
"""CausalPrefixAttention Trainium2 Bass kernel.

Sharding: core = 4*batch + head_group. Each core computes, for its batch b and
its 4 heads, the full pipeline LN(x), LN(context) -> q/k/v projections ->
causal-prefix attention -> out @ Wo_slice, producing a [2048, 1024] partial.
Host sums the 4 partials per batch (row-parallel Wo) and adds bo.
"""

import sys

import numpy as np

for _p in ("/opt/trn_rl_repo", "/root/.axon_site/_ro/trn_rl_repo"):
    if _p not in sys.path:
        sys.path.append(_p)

import ml_dtypes  # noqa: E402

import concourse.bass as bass  # noqa: E402
import concourse.mybir as mybir  # noqa: E402
import concourse.tile as tile  # noqa: E402
from concourse import bacc  # noqa: E402
from concourse.bass_utils import run_bass_kernel_spmd  # noqa: E402

BF16 = mybir.dt.bfloat16
F32 = mybir.dt.float32
F32R = mybir.dt.float32r

N = 2048          # query tokens per batch
CTX = 2048        # context tokens per batch
DIM = 1024
DH = 64           # head dim
HPC = 4           # heads per core
CPC = HPC * DH    # 256 inner cols per core
J = CTX + N       # 4096 total keys
EPS = 1e-5

AF = mybir.ActivationFunctionType
ALU = mybir.AluOpType


def build_nc() -> bass.Bass:
    nc = bacc.Bacc()

    xb = nc.declare_dram_parameter("xb", [N, DIM], BF16, isOutput=False)
    cb = nc.declare_dram_parameter("cb", [CTX, DIM], BF16, isOutput=False)
    wq = nc.declare_dram_parameter("wq", [DIM, CPC], BF16, isOutput=False)
    wk = nc.declare_dram_parameter("wk", [2, DIM, CPC], BF16, isOutput=False)
    wv = nc.declare_dram_parameter("wv", [2, DIM, CPC], BF16, isOutput=False)
    wo = nc.declare_dram_parameter("wo", [64, HPC, DIM], BF16, isOutput=False)
    cbq = nc.declare_dram_parameter("cbq", [128, 2], F32, isOutput=False)
    cbk = nc.declare_dram_parameter("cbk", [128, 2, 2], F32, isOutput=False)
    vbv = nc.declare_dram_parameter("vbv", [128, 2, HPC, 64], BF16,
                                    isOutput=False)
    cmv = nc.declare_dram_parameter("cmv", [128, 16], F32, isOutput=False)
    tri01 = nc.declare_dram_parameter("tri01", [128, 128], BF16, isOutput=False)
    ident = nc.declare_dram_parameter("ident", [128, 128], BF16, isOutput=False)
    sel16 = nc.declare_dram_parameter("sel16", [16, 2048], F32, isOutput=False)
    out_d = nc.declare_dram_parameter("out", [N, DIM], F32, isOutput=True)

    with tile.TileContext(nc) as tc:
        with (
            tc.tile_pool(name="singles", bufs=1) as singles,
            tc.tile_pool(name="acts", bufs=1) as acts,
            tc.tile_pool(name="ln", bufs=3) as ln_pool,
            tc.tile_pool(name="lns", bufs=4) as lns,
            tc.tile_pool(name="es", bufs=4) as es_pool,
            tc.tile_pool(name="dstg", bufs=3) as dstg_pool,
            tc.tile_pool(name="outp", bufs=2) as out_pool,
            tc.tile_pool(name="ps", bufs=2, space="PSUM") as psum,
            tc.tile_pool(name="pj", bufs=1, space="PSUM") as pj_pool,
            tc.tile_pool(name="dnp", bufs=1, space="PSUM") as den_pool,
            tc.tile_pool(name="avps", bufs=2, space="PSUM") as av_pool,
        ):
            # --- constants / weights to SBUF ---
            wq_sb = singles.tile([128, 8, CPC], BF16)
            nc.sync.dma_start(wq_sb, wq.rearrange("(t p) c -> p t c", p=128))
            wk_sb = singles.tile([128, 2, 8, CPC], BF16)
            nc.sync.dma_start(wk_sb, wk.rearrange("s (t p) c -> p s t c", p=128))
            wv_sb = singles.tile([128, 2, 8, CPC], BF16)
            nc.sync.dma_start(wv_sb, wv.rearrange("s (t p) c -> p s t c", p=128))
            wo_sb = singles.tile([64, HPC, DIM], BF16)
            nc.sync.dma_start(wo_sb, wo[:])
            scol = singles.tile([128, 16, 16], F32)
            nc.vector.memset(scol, 0.0)
            for r in range(16):
                nc.vector.memset(scol[64:65, r, r:r + 1], 1.0)
            cbq_sb = singles.tile([128, 2], F32)
            nc.sync.dma_start(cbq_sb, cbq[:])
            cbk_sb = singles.tile([128, 2, 2], F32)
            nc.sync.dma_start(cbk_sb, cbk[:])
            vb_sb = singles.tile([128, 2, HPC, 64], BF16)
            nc.sync.dma_start(vb_sb, vbv[:])
            cm_sb = singles.tile([128, 16], F32)
            nc.sync.dma_start(cm_sb, cmv[:])
            tri_sb = singles.tile([128, 128], BF16)
            nc.sync.dma_start(tri_sb, tri01[:])
            id_sb = singles.tile([128, 128], BF16)
            nc.sync.dma_start(id_sb, ident[:])
            sel_sb = singles.tile([16, 2048], F32)
            nc.sync.dma_start(sel_sb, sel16[:])
            eps_sb = singles.tile([128, 1], F32)
            nc.vector.memset(eps_sb, EPS)

            # --- LayerNorm + transpose: [tokens, DIM] -> [128, 8, tokens] ---
            # stats on ScalarE via accum_out; apply + transpose-copy on DVE
            def layernorm_T(src_dram, ntok, name):
                dstT = acts.tile([128, 8, ntok], BF16, tag=f"T{name}")
                for rt in range(ntok // 128):
                    xt = ln_pool.tile([128, DIM], BF16, tag="xt")
                    nc.sync.dma_start(xt, src_dram[rt * 128:(rt + 1) * 128, :])
                    st = lns.tile([128, 2, 6], F32, tag="st")
                    nc.vector.bn_stats(st[:, 0, :], xt[:, 0:512])
                    nc.vector.bn_stats(st[:, 1, :], xt[:, 512:1024])
                    mv = lns.tile([128, 2], F32, tag="mv")
                    nc.vector.bn_aggr(mv, st)
                    std = lns.tile([128, 1], F32, tag="std")
                    nc.scalar.activation(std, mv[:, 1:2], AF.Sqrt, bias=eps_sb)
                    rstd = lns.tile([128, 1], F32, tag="rstd")
                    nc.vector.reciprocal(rstd, std)
                    xn = ln_pool.tile([128, DIM], BF16, tag="xn")
                    nc.vector.tensor_scalar(
                        xn, xt, mv[:, 0:1], rstd, op0=ALU.subtract, op1=ALU.mult
                    )
                    for fg in range(2):
                        pst = av_pool.tile([128, 512], BF16, tag="av")
                        for k in range(4):
                            ft = fg * 4 + k
                            nc.tensor.transpose(
                                pst[:, k * 128:(k + 1) * 128],
                                xn[:, ft * 128:(ft + 1) * 128],
                                id_sb,
                            )
                        nc.scalar.copy(
                            dstT[:, fg * 4:(fg + 1) * 4, rt * 128:(rt + 1) * 128],
                            pst.rearrange("p (f c) -> p f c", f=4),
                        )
                return dstT

            xnT = layernorm_T(xb, N, "x")
            cnT = layernorm_T(cb, CTX, "c")

            # --- projections, split per c-tile so attention on heads 0/1 can
            # overlap with the projections for heads 2/3 ---
            def make_qT(ct):
                chunks = []
                for it in range(N // 1024):
                    qT = acts.tile([128, 1024], BF16, tag=f"qT{ct}_{it}",
                                   name=f"qT{ct}_{it}")
                    for half in range(2):
                        ps = pj_pool.tile([128, 512], F32, tag="pj")
                        off = it * 1024 + half * 512
                        for kt in range(8):
                            nc.tensor.matmul(
                                ps,
                                wq_sb[:, kt, ct * 128:(ct + 1) * 128],
                                xnT[:, kt, off:off + 512],
                                start=(kt == 0), stop=(kt == 7),
                            )
                        nc.vector.tensor_scalar_add(
                            qT[:, half * 512:(half + 1) * 512], ps,
                            cbq_sb[:, ct:ct + 1])
                    chunks.append(qT)
                return chunks

            def make_kT(ct):
                chunks = []
                for jt in range(J // 1024):
                    kT = acts.tile([128, 1024], BF16, tag=f"kT{ct}_{jt}",
                                   name=f"kT{ct}_{jt}")
                    for half in range(2):
                        ps = pj_pool.tile([128, 512], F32, tag="pj")
                        j5 = jt * 2 + half
                        s = 0 if j5 < 4 else 1
                        srcT = cnT if j5 < 4 else xnT
                        off = (j5 % 4) * 512
                        for kt in range(8):
                            nc.tensor.matmul(
                                ps,
                                wk_sb[:, s, kt, ct * 128:(ct + 1) * 128],
                                srcT[:, kt, off:off + 512],
                                start=(kt == 0), stop=(kt == 7),
                            )
                        nc.vector.tensor_scalar_add(
                            kT[:, half * 512:(half + 1) * 512], ps,
                            cbk_sb[:, s, ct:ct + 1])
                    chunks.append(kT)
                return chunks

            qTs = {0: make_qT(0)}
            kTs = {0: make_kT(0)}

            # --- v natural [keys, 4 heads, 64+aug] ---
            v_tiles = []
            for jb in range(32):
                vt = acts.tile([128, HPC, 66], BF16, tag=f"v{jb}",
                               name=f"v{jb}")
                v_tiles.append(vt)
                s = 0 if jb < 16 else 1
                srcT = cnT if jb < 16 else xnT
                off = (jb % 16) * 128
                ps = pj_pool.tile([128, 512], F32, tag="pj")
                for kt in range(8):
                    nc.tensor.matmul(
                        ps[:, 0:CPC],
                        srcT[:, kt, off:off + 128],
                        wv_sb[:, s, kt, :],
                        start=(kt == 0), stop=(kt == 7),
                    )
                nc.vector.tensor_add(
                    vt[:, :, 0:64],
                    ps[:, 0:CPC].rearrange("p (h d) -> p h d", h=HPC),
                    vb_sb[:, s, :, :],
                )
                if jb < 16:
                    # context_mask: zero masked rows, aug col = mask
                    nc.vector.tensor_scalar_mul(
                        vt[:, :, 0:64], vt[:, :, 0:64],
                        cm_sb[:, jb:jb + 1],
                    )
                    nc.vector.tensor_copy(
                        vt[:, :, 64:65],
                        cm_sb[:, jb:jb + 1, None].to_broadcast((128, HPC, 1)),
                    )
                else:
                    nc.vector.memset(vt[:, :, 64:65], 1.0)

            # --- attention ---
            rden = singles.tile([16, 512], F32)
            # out^T as 16 separate tiles (head, 512-query block) so the out
            # projection can start per-block as soon as normalization lands
            oThs = {}
            for h in range(HPC):
                for q in range(4):
                    oThs[(h, q)] = acts.tile([128, 512], BF16, tag=f"oT{h}_{q}", name=f"oT{h}_{q}")
            den_acc = den_pool.tile([8, 512], F32)
            n_den = [0, 0]

            def attend(h):
                ct, pb = h // 2, (h % 2) * 64
                kT, qT = kTs[ct], qTs[ct]
                for it in range(2):
                    i0 = it * 1024
                    njs = 16 + it * 8 + 8
                    jl0 = [j for j in range(njs)
                           if j < 16 or (j - 16) * 128 - i0 < 512]
                    jl1 = list(range(njs))
                    av0 = av_pool.tile([128, 512], F32, tag="av")
                    av1 = av_pool.tile([128, 512], F32, tag="av")

                    def emit_av(jb, es):
                        if jb in jl0:
                            nc.tensor.matmul(
                                av0[0:65, :],
                                v_tiles[jb][:, h, 0:65],
                                es[:, 0:512],
                                start=(jb == jl0[0]), stop=(jb == jl0[-1]),
                            )
                        nc.tensor.matmul(
                            av1[0:65, :],
                            v_tiles[jb][:, h, 0:65],
                            es[:, 512:1024],
                            start=(jb == jl1[0]), stop=(jb == jl1[-1]),
                        )

                    pending = None  # (jb, es) awaiting AV, lags one j-block
                    for jb in range(njs):
                        jj0 = (jb - 16) * 128
                        d = jj0 - i0
                        crossing = jb >= 16 and d >= 0
                        c0 = d if (crossing and d > 0) else 0
                        kc = kT[jb // 8][pb:pb + 64,
                                         (jb % 8) * 128:(jb % 8 + 1) * 128]
                        qc = qT[it]
                        ps = psum.tile([128, 1024], F32, tag="ps")
                        if c0 < 512:
                            nc.tensor.matmul(
                                ps[:, c0:512],
                                kc,
                                qc[pb:pb + 64, c0:512],
                                start=True, stop=True,
                            )
                        nc.tensor.matmul(
                            ps[:, max(512, c0):1024],
                            kc,
                            qc[pb:pb + 64, max(512, c0):1024],
                            start=True, stop=True,
                        )
                        es = es_pool.tile([128, 1024], BF16, tag="es")
                        if c0 > 0:
                            nc.gpsimd.memset(es[:, 0:c0], 0.0)
                        nc.scalar.activation(
                            es[:, c0:1024], ps[:, c0:1024], AF.Exp)
                        if crossing:
                            nc.gpsimd.tensor_mul(
                                es[:, d:d + 128], es[:, d:d + 128], tri_sb
                            )
                        if pending is not None:
                            emit_av(*pending)
                        pending = (jb, es)
                    emit_av(*pending)
                    for half, av in ((0, av0), (1, av1)):
                        i5 = i0 + half * 512
                        nc.vector.tensor_copy(
                            oThs[(h, i5 // 512)][0:64, :], av[0:64, :])
                        dstg = dstg_pool.tile([128, 512], F32, tag="dstg")
                        nc.vector.tensor_copy(dstg[64:65, :], av[64:65, :])
                        r = h * 4 + it * 2 + half
                        grp = r // 8
                        n_den[grp] += 1
                        nc.tensor.matmul(
                            den_acc,
                            scol[64:65, r, grp * 8:(grp + 1) * 8],
                            dstg[64:65, :],
                            start=(n_den[grp] == 1), stop=(n_den[grp] == 8),
                        )

            def normalize(heads, rd):
                for it in range(4):
                    for h in heads:
                        r = (h % 2) * 4 + it
                        bc = av_pool.tile([128, 512], F32, tag="av")
                        nc.tensor.matmul(
                            bc,
                            sel_sb[0:8, r * 128:(r + 1) * 128],
                            rd,
                            start=True, stop=True,
                        )
                        nc.vector.tensor_mul(
                            oThs[(h, it)][0:64, :],
                            oThs[(h, it)][0:64, :], bc[0:64, :],
                        )

            attend(0)
            qTs[1] = make_qT(1)
            kTs[1] = make_kT(1)
            attend(1)
            nc.vector.reciprocal(rden[0:8, :], den_acc)
            normalize([0, 1], rden[0:8, :])
            attend(2)
            attend(3)
            rden2 = singles.tile([8, 512], F32)
            nc.vector.reciprocal(rden2[0:8, :], den_acc)
            normalize([2, 3], rden2[0:8, :])

            # --- out projection: out[i, :] = oTh^T @ wo (4 heads, K=64) ---
            for ib in range(N // 128):
                ot = out_pool.tile([128, DIM], F32, tag="ot")
                ps = psum.tile([128, 1024], F32, tag="ps")
                for oc in range(2):
                    for h in range(HPC):
                        nc.tensor.matmul(
                            ps[:, oc * 512:(oc + 1) * 512],
                            oThs[(h, ib // 4)][0:64,
                                               (ib % 4) * 128:(ib % 4 + 1) * 128],
                            wo_sb[:, h, oc * 512:(oc + 1) * 512],
                            start=(h == 0), stop=(h == 3),
                        )
                nc.scalar.copy(ot, ps)
                nc.sync.dma_start(out_d[ib * 128:(ib + 1) * 128, :], ot)

    nc.finalize()
    return nc


def _sel16():
    s = np.zeros((16, 2048), np.float32)
    for r in range(16):
        s[r, r * 128:(r + 1) * 128] = 1.0
    return s


def make_in_maps(x, context, context_mask, g1, b1, g2, b2, Wq, Wkv, Wo):
    bf = ml_dtypes.bfloat16
    Wk = Wkv[:, :DIM]
    Wv = Wkv[:, DIM:]
    scale = DH ** -0.5
    tri = np.triu(np.ones((128, 128), np.float32)).astype(bf)
    g1 = np.asarray(g1, np.float32)
    g2 = np.asarray(g2, np.float32)
    b1 = np.asarray(b1, np.float32)
    b2 = np.asarray(b2, np.float32)

    in_maps = []
    for core in range(8):
        b, g = core // 4, core % 4
        hs = slice(g * CPC, (g + 1) * CPC)
        wq_g = g1[:, None] * Wq[:, hs] * scale
        # source 0 = context (g2/b2), source 1 = self (g1/b1)
        wk2 = np.stack([g2[:, None] * Wk[:, hs], g1[:, None] * Wk[:, hs]])
        wv2 = np.stack([g2[:, None] * Wv[:, hs], g1[:, None] * Wv[:, hs]])
        cbq_a = (b1 @ Wq[:, hs]) * scale          # [256]
        cbk_a = np.stack([b2 @ Wk[:, hs], b1 @ Wk[:, hs]])   # [2, 256]
        vb_a = np.stack([b2 @ Wv[:, hs], b1 @ Wv[:, hs]])    # [2, 256]
        in_maps.append(dict(
            xb=np.ascontiguousarray(x[b]).astype(bf),
            cb=np.ascontiguousarray(context[b]).astype(bf),
            wq=np.ascontiguousarray(wq_g).astype(bf),
            wk=np.ascontiguousarray(wk2).astype(bf),
            wv=np.ascontiguousarray(wv2).astype(bf),
            wo=np.ascontiguousarray(
                Wo[hs, :].reshape(HPC, 64, DIM).transpose(1, 0, 2)
            ).astype(bf),
            cbq=np.ascontiguousarray(cbq_a.reshape(2, 128).T),
            cbk=np.ascontiguousarray(
                cbk_a.reshape(2, 2, 128).transpose(2, 0, 1)),
            vbv=np.ascontiguousarray(np.broadcast_to(
                vb_a.reshape(1, 2, HPC, 64), (128, 2, HPC, 64))).astype(bf),
            cmv=np.ascontiguousarray(
                np.asarray(context_mask[b], np.float32).reshape(16, 128).T
            ),
            tri01=tri,
            ident=np.eye(128, dtype=np.float32).astype(bf),
            sel16=_sel16(),
        ))
    return in_maps


_NC_CACHE = None


def kernel(**inputs) -> np.ndarray:
    global _NC_CACHE
    x = np.asarray(inputs["x"], np.float32)
    context = np.asarray(inputs["context"], np.float32)
    cm = np.asarray(inputs["context_mask"])
    g1 = np.asarray(inputs["g1"], np.float32)
    b1 = np.asarray(inputs["b1"], np.float32)
    g2 = np.asarray(inputs["g2"], np.float32)
    b2 = np.asarray(inputs["b2"], np.float32)
    Wq = np.asarray(inputs["Wq"], np.float32)
    Wkv = np.asarray(inputs["Wkv"], np.float32)
    Wo = np.asarray(inputs["Wo"], np.float32)
    bo = np.asarray(inputs["bo"], np.float32)

    if _NC_CACHE is None:
        _NC_CACHE = build_nc()
    nc = _NC_CACHE

    # The SPMD run dispatches through jax/PJRT on the axon backend; if the
    # caller pinned jax to cpu (common for reference computation), restore
    # the full platform list so the 8 NeuronCores are visible.
    import jax
    if len(jax.devices()) < 8:
        import os
        os.environ.pop("JAX_PLATFORMS", None)
        try:
            jax.config.update("jax_platforms", None)
        except Exception:
            pass
        try:
            from jax.extend import backend as _jxb
            _jxb.clear_backends()
        except Exception:
            from jax._src import xla_bridge as _xb
            _xb.backends.cache_clear()

    in_maps = make_in_maps(x, context, cm, g1, b1, g2, b2, Wq, Wkv, Wo)
    res = run_bass_kernel_spmd(nc, in_maps, core_ids=list(range(8))).results

    out = np.zeros((2, N, DIM), np.float32)
    for core in range(8):
        out[core // 4] += np.asarray(res[core]["out"], np.float32)
    out += bo
    return out


# revision 43
# speedup vs baseline: 1.1234x; 1.0780x over previous
"""CausalPrefixAttention Trainium2 Bass kernel.

Sharding: core = 4*batch + head_group. Each core computes, for its batch b and
its 4 heads, the full pipeline LN(x), LN(context) -> q/k/v projections ->
causal-prefix attention -> out @ Wo_slice, producing a [2048, 1024] partial.
Host sums the 4 partials per batch (row-parallel Wo) and adds bo.
"""

import sys

import numpy as np

for _p in ("/opt/trn_rl_repo", "/root/.axon_site/_ro/trn_rl_repo"):
    if _p not in sys.path:
        sys.path.append(_p)

import ml_dtypes  # noqa: E402

import concourse.bass as bass  # noqa: E402
import concourse.mybir as mybir  # noqa: E402
import concourse.tile as tile  # noqa: E402
from concourse import bacc  # noqa: E402
from concourse.bass_utils import run_bass_kernel_spmd  # noqa: E402

BF16 = mybir.dt.bfloat16
F32 = mybir.dt.float32
F32R = mybir.dt.float32r

N = 2048          # query tokens per batch
CTX = 2048        # context tokens per batch
DIM = 1024
DH = 64           # head dim
HPC = 4           # heads per core
CPC = HPC * DH    # 256 inner cols per core
J = CTX + N       # 4096 total keys
EPS = 1e-5

AF = mybir.ActivationFunctionType
ALU = mybir.AluOpType


def build_nc() -> bass.Bass:
    nc = bacc.Bacc()

    xb = nc.declare_dram_parameter("xb", [N, DIM], BF16, isOutput=False)
    cb = nc.declare_dram_parameter("cb", [CTX, DIM], BF16, isOutput=False)
    wq = nc.declare_dram_parameter("wq", [DIM, CPC], BF16, isOutput=False)
    wk = nc.declare_dram_parameter("wk", [2, DIM, CPC], BF16, isOutput=False)
    wv = nc.declare_dram_parameter("wv", [2, DIM, CPC], BF16, isOutput=False)
    wo = nc.declare_dram_parameter("wo", [64, HPC, DIM], BF16, isOutput=False)
    cbq = nc.declare_dram_parameter("cbq", [128, 2], F32, isOutput=False)
    cbk = nc.declare_dram_parameter("cbk", [128, 2, 2], F32, isOutput=False)
    vbv = nc.declare_dram_parameter("vbv", [128, 2, HPC, 64], BF16,
                                    isOutput=False)
    cmv = nc.declare_dram_parameter("cmv", [128, 16], F32, isOutput=False)
    tri01 = nc.declare_dram_parameter("tri01", [128, 128], BF16, isOutput=False)
    ident = nc.declare_dram_parameter("ident", [128, 128], BF16, isOutput=False)
    sel16 = nc.declare_dram_parameter("sel16", [16, 2048], F32, isOutput=False)
    out_d = nc.declare_dram_parameter("out", [N, DIM], F32, isOutput=True)

    with tile.TileContext(nc) as tc:
        with (
            tc.tile_pool(name="singles", bufs=1) as singles,
            tc.tile_pool(name="acts", bufs=1) as acts,
            tc.tile_pool(name="ln", bufs=3) as ln_pool,
            tc.tile_pool(name="lns", bufs=4) as lns,
            tc.tile_pool(name="es", bufs=4) as es_pool,
            tc.tile_pool(name="dstg", bufs=3) as dstg_pool,
            tc.tile_pool(name="outp", bufs=2) as out_pool,
            tc.tile_pool(name="ps", bufs=2, space="PSUM") as psum,
            tc.tile_pool(name="pj", bufs=1, space="PSUM") as pj_pool,
            tc.tile_pool(name="dnp", bufs=1, space="PSUM") as den_pool,
            tc.tile_pool(name="avps", bufs=2, space="PSUM") as av_pool,
        ):
            # --- constants / weights to SBUF ---
            wq_sb = singles.tile([128, 8, CPC], BF16)
            nc.sync.dma_start(wq_sb, wq.rearrange("(t p) c -> p t c", p=128))
            wk_sb = singles.tile([128, 2, 8, CPC], BF16)
            nc.sync.dma_start(wk_sb, wk.rearrange("s (t p) c -> p s t c", p=128))
            wv_sb = singles.tile([128, 2, 8, CPC], BF16)
            nc.sync.dma_start(wv_sb, wv.rearrange("s (t p) c -> p s t c", p=128))
            wo_sb = singles.tile([64, HPC, DIM], BF16)
            nc.sync.dma_start(wo_sb, wo[:])
            scol = singles.tile([128, 16, 16], F32)
            nc.vector.memset(scol, 0.0)
            for r in range(16):
                nc.vector.memset(scol[64:65, r, r:r + 1], 1.0)
            cbq_sb = singles.tile([128, 2], F32)
            nc.sync.dma_start(cbq_sb, cbq[:])
            cbk_sb = singles.tile([128, 2, 2], F32)
            nc.sync.dma_start(cbk_sb, cbk[:])
            vb_sb = singles.tile([128, 2, HPC, 64], BF16)
            nc.sync.dma_start(vb_sb, vbv[:])
            cm_sb = singles.tile([128, 16], F32)
            nc.sync.dma_start(cm_sb, cmv[:])
            tri_sb = singles.tile([128, 128], BF16)
            nc.sync.dma_start(tri_sb, tri01[:])
            id_sb = singles.tile([128, 128], BF16)
            nc.sync.dma_start(id_sb, ident[:])
            sel_sb = singles.tile([16, 2048], F32)
            nc.sync.dma_start(sel_sb, sel16[:])
            eps_sb = singles.tile([128, 1], F32)
            nc.vector.memset(eps_sb, EPS)

            # --- LayerNorm + transpose: [tokens, DIM] -> [128, 8, tokens] ---
            # stats on ScalarE via accum_out; apply + transpose-copy on DVE
            def layernorm_T(src_dram, ntok, name):
                dstT = acts.tile([128, 8, ntok], BF16, tag=f"T{name}")
                for rt in range(ntok // 128):
                    xt = ln_pool.tile([128, DIM], BF16, tag="xt")
                    nc.sync.dma_start(xt, src_dram[rt * 128:(rt + 1) * 128, :])
                    st = lns.tile([128, 2, 6], F32, tag="st")
                    nc.vector.bn_stats(st[:, 0, :], xt[:, 0:512])
                    nc.vector.bn_stats(st[:, 1, :], xt[:, 512:1024])
                    mv = lns.tile([128, 2], F32, tag="mv")
                    nc.vector.bn_aggr(mv, st)
                    std = lns.tile([128, 1], F32, tag="std")
                    nc.scalar.activation(std, mv[:, 1:2], AF.Sqrt, bias=eps_sb)
                    rstd = lns.tile([128, 1], F32, tag="rstd")
                    nc.vector.reciprocal(rstd, std)
                    xn = ln_pool.tile([128, DIM], BF16, tag="xn")
                    nc.vector.tensor_scalar(
                        xn, xt, mv[:, 0:1], rstd, op0=ALU.subtract, op1=ALU.mult
                    )
                    for fg in range(2):
                        pst = av_pool.tile([128, 512], BF16, tag="av")
                        for k in range(4):
                            ft = fg * 4 + k
                            nc.tensor.transpose(
                                pst[:, k * 128:(k + 1) * 128],
                                xn[:, ft * 128:(ft + 1) * 128],
                                id_sb,
                            )
                        nc.scalar.copy(
                            dstT[:, fg * 4:(fg + 1) * 4, rt * 128:(rt + 1) * 128],
                            pst.rearrange("p (f c) -> p f c", f=4),
                        )
                return dstT

            xnT = layernorm_T(xb, N, "x")
            cnT = layernorm_T(cb, CTX, "c")

            # --- projections, split per c-tile so attention on heads 0/1 can
            # overlap with the projections for heads 2/3 ---
            def make_qT(ct):
                chunks = []
                for it in range(N // 1024):
                    qT = acts.tile([128, 1024], BF16, tag=f"qT{ct}_{it}",
                                   name=f"qT{ct}_{it}")
                    for half in range(2):
                        ps = pj_pool.tile([128, 512], F32, tag="pj")
                        off = it * 1024 + half * 512
                        for kt in range(8):
                            nc.tensor.matmul(
                                ps,
                                wq_sb[:, kt, ct * 128:(ct + 1) * 128],
                                xnT[:, kt, off:off + 512],
                                start=(kt == 0), stop=(kt == 7),
                            )
                        nc.vector.tensor_scalar_add(
                            qT[:, half * 512:(half + 1) * 512], ps,
                            cbq_sb[:, ct:ct + 1])
                    chunks.append(qT)
                return chunks

            def make_kT(ct):
                chunks = []
                for jt in range(J // 1024):
                    kT = acts.tile([128, 1024], BF16, tag=f"kT{ct}_{jt}",
                                   name=f"kT{ct}_{jt}")
                    for half in range(2):
                        ps = pj_pool.tile([128, 512], F32, tag="pj")
                        j5 = jt * 2 + half
                        s = 0 if j5 < 4 else 1
                        srcT = cnT if j5 < 4 else xnT
                        off = (j5 % 4) * 512
                        for kt in range(8):
                            nc.tensor.matmul(
                                ps,
                                wk_sb[:, s, kt, ct * 128:(ct + 1) * 128],
                                srcT[:, kt, off:off + 512],
                                start=(kt == 0), stop=(kt == 7),
                            )
                        nc.vector.tensor_scalar_add(
                            kT[:, half * 512:(half + 1) * 512], ps,
                            cbk_sb[:, s, ct:ct + 1])
                    chunks.append(kT)
                return chunks

            qTs = {0: make_qT(0)}
            kTs = {0: make_kT(0)}

            # --- v natural [keys, 4 heads, 64+aug] ---
            v_tiles = []
            for jb in range(32):
                vt = acts.tile([128, HPC, 66], BF16, tag=f"v{jb}",
                               name=f"v{jb}")
                v_tiles.append(vt)
                s = 0 if jb < 16 else 1
                srcT = cnT if jb < 16 else xnT
                off = (jb % 16) * 128
                ps = pj_pool.tile([128, 512], F32, tag="pj")
                for kt in range(8):
                    nc.tensor.matmul(
                        ps[:, 0:CPC],
                        srcT[:, kt, off:off + 128],
                        wv_sb[:, s, kt, :],
                        start=(kt == 0), stop=(kt == 7),
                    )
                nc.vector.tensor_add(
                    vt[:, :, 0:64],
                    ps[:, 0:CPC].rearrange("p (h d) -> p h d", h=HPC),
                    vb_sb[:, s, :, :],
                )
                if jb < 16:
                    # context_mask: zero masked rows, aug col = mask
                    nc.vector.tensor_scalar_mul(
                        vt[:, :, 0:64], vt[:, :, 0:64],
                        cm_sb[:, jb:jb + 1],
                    )
                    nc.vector.tensor_copy(
                        vt[:, :, 64:65],
                        cm_sb[:, jb:jb + 1, None].to_broadcast((128, HPC, 1)),
                    )
                else:
                    nc.vector.memset(vt[:, :, 64:65], 1.0)

            # --- attention ---
            rden = singles.tile([16, 512], F32)
            # out^T as 16 separate tiles (head, 512-query block) so the out
            # projection can start per-block as soon as normalization lands
            oThs = {}
            for h in range(HPC):
                for q in range(4):
                    oThs[(h, q)] = acts.tile([128, 512], BF16, tag=f"oT{h}_{q}", name=f"oT{h}_{q}")
            den_acc = den_pool.tile([8, 512], F32)
            n_den = [0, 0]

            def attend(h):
                ct, pb = h // 2, (h % 2) * 64
                kT, qT = kTs[ct], qTs[ct]
                for it in range(2):
                    i0 = it * 1024
                    njs = 16 + it * 8 + 8
                    jl0 = [j for j in range(njs)
                           if j < 16 or (j - 16) * 128 - i0 < 512]
                    jl1 = list(range(njs))
                    av0 = av_pool.tile([128, 512], F32, tag="av")
                    av1 = av_pool.tile([128, 512], F32, tag="av")

                    def emit_av(jb, es):
                        if jb in jl0:
                            nc.tensor.matmul(
                                av0[0:65, :],
                                v_tiles[jb][:, h, 0:65],
                                es[:, 0:512],
                                start=(jb == jl0[0]), stop=(jb == jl0[-1]),
                            )
                        nc.tensor.matmul(
                            av1[0:65, :],
                            v_tiles[jb][:, h, 0:65],
                            es[:, 512:1024],
                            start=(jb == jl1[0]), stop=(jb == jl1[-1]),
                        )

                    pending = None  # (jb, es) awaiting AV, lags one j-block
                    for jb in range(njs):
                        jj0 = (jb - 16) * 128
                        d = jj0 - i0
                        crossing = jb >= 16 and d >= 0
                        c0 = d if (crossing and d > 0) else 0
                        kc = kT[jb // 8][pb:pb + 64,
                                         (jb % 8) * 128:(jb % 8 + 1) * 128]
                        qc = qT[it]
                        ps = psum.tile([128, 1024], F32, tag="ps")
                        if c0 < 512:
                            nc.tensor.matmul(
                                ps[:, c0:512],
                                kc,
                                qc[pb:pb + 64, c0:512],
                                start=True, stop=True,
                            )
                        nc.tensor.matmul(
                            ps[:, max(512, c0):1024],
                            kc,
                            qc[pb:pb + 64, max(512, c0):1024],
                            start=True, stop=True,
                        )
                        es = es_pool.tile([128, 1024], BF16, tag="es")
                        if c0 > 0:
                            nc.gpsimd.memset(es[:, 0:c0], 0.0)
                        nc.scalar.activation(
                            es[:, c0:1024], ps[:, c0:1024], AF.Exp)
                        if crossing:
                            nc.gpsimd.tensor_mul(
                                es[:, d:d + 128], es[:, d:d + 128], tri_sb
                            )
                        if pending is not None:
                            emit_av(*pending)
                        pending = (jb, es)
                    emit_av(*pending)
                    for half, av in ((0, av0), (1, av1)):
                        i5 = i0 + half * 512
                        nc.vector.tensor_copy(
                            oThs[(h, i5 // 512)][0:64, :], av[0:64, :])
                        dstg = dstg_pool.tile([128, 512], F32, tag="dstg")
                        nc.vector.tensor_copy(dstg[64:65, :], av[64:65, :])
                        r = h * 4 + it * 2 + half
                        grp = r // 8
                        n_den[grp] += 1
                        nc.tensor.matmul(
                            den_acc,
                            scol[64:65, r, grp * 8:(grp + 1) * 8],
                            dstg[64:65, :],
                            start=(n_den[grp] == 1), stop=(n_den[grp] == 8),
                        )

            def normalize(heads, rd):
                for it in range(4):
                    for h in heads:
                        r = (h % 2) * 4 + it
                        bc = av_pool.tile([128, 512], F32, tag="av")
                        nc.tensor.matmul(
                            bc,
                            sel_sb[0:8, r * 128:(r + 1) * 128],
                            rd,
                            start=True, stop=True,
                        )
                        nc.vector.tensor_mul(
                            oThs[(h, it)][0:64, :],
                            oThs[(h, it)][0:64, :], bc[0:64, :],
                        )

            attend(0)
            qTs[1] = make_qT(1)
            kTs[1] = make_kT(1)
            attend(1)
            nc.vector.reciprocal(rden[0:8, :], den_acc)
            normalize([0, 1], rden[0:8, :])
            attend(2)
            attend(3)
            rden2 = singles.tile([8, 512], F32)
            nc.vector.reciprocal(rden2[0:8, :], den_acc)
            normalize([2, 3], rden2[0:8, :])

            # --- out projection: out[i, :] = oTh^T @ wo (4 heads, K=64) ---
            for ib in range(N // 128):
                ot = out_pool.tile([128, DIM], F32, tag="ot")
                ps = psum.tile([128, 1024], F32, tag="ps")
                for oc in range(2):
                    for h in range(HPC):
                        nc.tensor.matmul(
                            ps[:, oc * 512:(oc + 1) * 512],
                            oThs[(h, ib // 4)][0:64,
                                               (ib % 4) * 128:(ib % 4 + 1) * 128],
                            wo_sb[:, h, oc * 512:(oc + 1) * 512],
                            start=(h == 0), stop=(h == 3),
                        )
                nc.scalar.copy(ot, ps)
                nc.sync.dma_start(out_d[ib * 128:(ib + 1) * 128, :], ot)

    nc.finalize()
    return nc


def _sel16():
    s = np.zeros((16, 2048), np.float32)
    for r in range(16):
        s[r, r * 128:(r + 1) * 128] = 1.0
    return s


def make_in_maps(x, context, context_mask, g1, b1, g2, b2, Wq, Wkv, Wo):
    bf = ml_dtypes.bfloat16
    Wk = Wkv[:, :DIM]
    Wv = Wkv[:, DIM:]
    scale = DH ** -0.5
    tri = np.triu(np.ones((128, 128), np.float32)).astype(bf)
    g1 = np.asarray(g1, np.float32)
    g2 = np.asarray(g2, np.float32)
    b1 = np.asarray(b1, np.float32)
    b2 = np.asarray(b2, np.float32)

    in_maps = []
    for core in range(8):
        b, g = core // 4, core % 4
        hs = slice(g * CPC, (g + 1) * CPC)
        wq_g = g1[:, None] * Wq[:, hs] * scale
        # source 0 = context (g2/b2), source 1 = self (g1/b1)
        wk2 = np.stack([g2[:, None] * Wk[:, hs], g1[:, None] * Wk[:, hs]])
        wv2 = np.stack([g2[:, None] * Wv[:, hs], g1[:, None] * Wv[:, hs]])
        cbq_a = (b1 @ Wq[:, hs]) * scale          # [256]
        cbk_a = np.stack([b2 @ Wk[:, hs], b1 @ Wk[:, hs]])   # [2, 256]
        vb_a = np.stack([b2 @ Wv[:, hs], b1 @ Wv[:, hs]])    # [2, 256]
        in_maps.append(dict(
            xb=np.ascontiguousarray(x[b]).astype(bf),
            cb=np.ascontiguousarray(context[b]).astype(bf),
            wq=np.ascontiguousarray(wq_g).astype(bf),
            wk=np.ascontiguousarray(wk2).astype(bf),
            wv=np.ascontiguousarray(wv2).astype(bf),
            wo=np.ascontiguousarray(
                Wo[hs, :].reshape(HPC, 64, DIM).transpose(1, 0, 2)
            ).astype(bf),
            cbq=np.ascontiguousarray(cbq_a.reshape(2, 128).T),
            cbk=np.ascontiguousarray(
                cbk_a.reshape(2, 2, 128).transpose(2, 0, 1)),
            vbv=np.ascontiguousarray(np.broadcast_to(
                vb_a.reshape(1, 2, HPC, 64), (128, 2, HPC, 64))).astype(bf),
            cmv=np.ascontiguousarray(
                np.asarray(context_mask[b], np.float32).reshape(16, 128).T
            ),
            tri01=tri,
            ident=np.eye(128, dtype=np.float32).astype(bf),
            sel16=_sel16(),
        ))
    return in_maps


_NC_CACHE = None


def kernel(**inputs) -> np.ndarray:
    global _NC_CACHE
    x = np.asarray(inputs["x"], np.float32)
    context = np.asarray(inputs["context"], np.float32)
    cm = np.asarray(inputs["context_mask"])
    g1 = np.asarray(inputs["g1"], np.float32)
    b1 = np.asarray(inputs["b1"], np.float32)
    g2 = np.asarray(inputs["g2"], np.float32)
    b2 = np.asarray(inputs["b2"], np.float32)
    Wq = np.asarray(inputs["Wq"], np.float32)
    Wkv = np.asarray(inputs["Wkv"], np.float32)
    Wo = np.asarray(inputs["Wo"], np.float32)
    bo = np.asarray(inputs["bo"], np.float32)

    if _NC_CACHE is None:
        _NC_CACHE = build_nc()
    nc = _NC_CACHE

    # The SPMD run dispatches through jax/PJRT on the axon backend; if the
    # caller pinned jax to cpu (common for reference computation), restore
    # the full platform list so the 8 NeuronCores are visible.
    import jax
    if len(jax.devices()) < 8:
        import os
        os.environ.pop("JAX_PLATFORMS", None)
        try:
            jax.config.update("jax_platforms", None)
        except Exception:
            pass
        try:
            from jax.extend import backend as _jxb
            _jxb.clear_backends()
        except Exception:
            from jax._src import xla_bridge as _xb
            _xb.backends.cache_clear()

    in_maps = make_in_maps(x, context, cm, g1, b1, g2, b2, Wq, Wkv, Wo)
    res = run_bass_kernel_spmd(nc, in_maps, core_ids=list(range(8))).results

    out = np.zeros((2, N, DIM), np.float32)
    for core in range(8):
        out[core // 4] += np.asarray(res[core]["out"], np.float32)
    out += bo
    return out
